# revision 19
# baseline (speedup 1.0000x reference)
"""Polyakov-loop generator kernel for Trainium2 (8 NeuronCores, SPMD).

Problem: U (complex 3x3 link field) on a [4, 24,24,24,24] lattice.
For each direction mu, every site x: P(x) = prod_{k=0..23} U_mu(x + k*mu_hat)
(periodic).  Output = stack([Re, Im]) of shape [2, 4, 24,24,24,24, 3, 3].

Strategy:
  - core c in 0..7 handles direction mu = c // 2, half h = c % 2 of the
    13824-site batch (the three non-mu lattice axes).
  - host canonicalizes U[mu] per core to [24 slices, 128 partitions, 486]
    (planar: 9 entries x 54 site-columns, entry-major), plus a precomputed
    U_re+U_im plane, so the SPMD program is direction-agnostic.
  - on-chip: all 24 cyclic products per line via prefix/suffix products:
        Q(s) = V0..V(s-1),  S(s) = Vs..V23,  P(s) = S(s) @ Q(s)
    68 batched complex-3x3 matmul steps instead of the naive 23*24.
  - each step is a Gauss-3M complex matmul (3 real bulk products, the 3
    contraction terms fused per instruction via broadcast access patterns),
    with (re+im) operand sums carried along the chains so most steps skip
    the 3M pre-additions.  All elementwise work runs on the Vector engine
    (concurrent GPSIMD measurably poisons DVE throughput via SBUF fabric
    contention).
"""

import sys

sys.path.insert(0, "/opt/trn_rl_repo")

import numpy as np

import concourse.bacc as bacc
import concourse.mybir as mybir
from concourse.tile import TileContext
from concourse.bass_utils import run_bass_kernel_spmd

F32 = mybir.dt.float32
L = 24          # lattice extent (product length)
NMU = 4
P = 128         # SBUF partitions
F = 54          # site-columns per partition (6912 = 128 * 54)
E = 9           # 3x3 entries
FB = F * E      # 486 free elems per slice
HALF = P * F    # 6912 sites per core
MULT = mybir.AluOpType.mult
ADD = mybir.AluOpType.add
SUB = mybir.AluOpType.subtract

# (engine_name, n_site_columns) per independent chain lane.
LANES = (("vector", 54),)

_prog_cache = {}


def _cmatmul(nc, eng, tpool, lane_id, n, cre, cim, are, aim, bre, bim,
             asum=None, bsum=None, csum=None):
    """C = A @ B (complex 3x3, batched) on packed [P, 9n] planar APs.

    Gauss 3M: M1 = Ar@Br, M2 = Ai@Bi, M3 = (Ar+Ai)@(Br+Bi);
    Cre = M1 - M2, Cim = M3 - M1 - M2.
    asum/bsum: optional precomputed (re+im) operand planes.
    csum: optional output AP to receive Cre+Cim (for chain carrying).
    """
    tt = eng.tensor_tensor
    jn = 27 * n
    w = E * n

    def av(x):  # A as [p, ij(9), k-bcast(3), s(n)]
        return (x.rearrange("p (ij s) -> p ij s", ij=9, s=n)
                .unsqueeze(2).broadcast_to([P, 9, 3, n]))

    def bv(x):  # B as [p, i-bcast(3), j(3), ks(3n)]
        return (x.rearrange("p (j ks) -> p j ks", j=3, ks=3 * n)
                .unsqueeze(1).broadcast_to([P, 3, 3, 3 * n]))

    def tv(t):  # product out view [p, ij(9), k(3), s(n)] (contiguous)
        return t.rearrange("p (ij k s) -> p ij k s", ij=9, k=3, s=n)

    def tj(t, j):  # j-th term [p, i(3), k(3), s(n)]
        return t.rearrange("p (i j ks) -> p i j ks",
                           i=3, j=3, ks=3 * n)[:, :, j, :]

    tab = tpool.tile([P, 2 * jn], F32, tag=f"tab{lane_id}",
                     name=f"tab{lane_id}")
    ta, tb = tab[:, :jn], tab[:, jn:]
    tc = tpool.tile([P, jn], F32, tag=f"tc{lane_id}", name=f"tc{lane_id}")
    m12 = tpool.tile([P, 2 * w], F32, tag=f"m12{lane_id}",
                     name=f"m12{lane_id}")
    m1, m2 = m12[:, :w], m12[:, w:]

    def tabj(j):  # j-th terms of both ta and tb: [p, ci(6), ks(3n)]
        return tab[:].rearrange("p (ci j ks) -> p ci j ks",
                                ci=6, j=3, ks=3 * n)[:, :, j, :]

    m12v = m12[:]

    if asum is None:
        at = tpool.tile([P, w], F32, tag=f"as{lane_id}", name=f"as{lane_id}")
        tt(out=at[:], in0=are, in1=aim, op=ADD)
        asum = at[:]
    if bsum is None:
        bt = tpool.tile([P, w], F32, tag=f"bs{lane_id}", name=f"bs{lane_id}")
        tt(out=bt[:], in0=bre, in1=bim, op=ADD)
        bsum = bt[:]

    tt(out=tv(ta), in0=av(are), in1=bv(bre), op=MULT)
    tt(out=tv(tb), in0=av(aim), in1=bv(bim), op=MULT)
    tt(out=tv(tc[:]), in0=av(asum), in1=bv(bsum), op=MULT)
    tt(out=m12v, in0=tabj(0), in1=tabj(1), op=ADD)      # [m1; m2] fold
    tt(out=m12v, in0=m12v, in1=tabj(2), op=ADD)
    tt(out=cim, in0=tj(tc[:], 0), in1=tj(tc[:], 1), op=ADD)
    tt(out=cim, in0=cim, in1=tj(tc[:], 2), op=ADD)
    tt(out=cre, in0=m1, in1=m2, op=SUB)
    tt(out=cim, in0=cim, in1=m1, op=SUB)
    tt(out=cim, in0=cim, in1=m2, op=SUB)
    if csum is not None:
        tt(out=csum, in0=cre, in1=cim, op=ADD)


def build_program():
    if "nc" in _prog_cache:
        return _prog_cache["nc"]

    nc = bacc.Bacc("TRN2", target_bir_lowering=False, debug=False, num_devices=8)
    ure_d = nc.declare_dram_parameter("u_re", [L, P, FB], F32, isOutput=False)
    uim_d = nc.declare_dram_parameter("u_im", [L, P, FB], F32, isOutput=False)
    usm_d = nc.declare_dram_parameter("u_sum", [L, P, FB], F32, isOutput=False)
    pre_d = nc.declare_dram_parameter("p_re", [L, P, FB], F32, isOutput=True)
    pim_d = nc.declare_dram_parameter("p_im", [L, P, FB], F32, isOutput=True)
    qsm_d = nc.dram_tensor("qsum_scratch", [L - 2, P, FB], F32)

    offs = []
    o = 0
    for _, n in LANES:
        offs.append(o)
        o += E * n
    assert o == FB

    with TileContext(nc) as tc:
        engs = {"vector": nc.vector, "gpsimd": nc.gpsimd}
        with (
            tc.tile_pool(name="qpool", bufs=1) as qpool,
            tc.tile_pool(name="upool", bufs=6) as upool,
            tc.tile_pool(name="spool", bufs=3) as spool,
            tc.tile_pool(name="ppool", bufs=6) as ppool,
            tc.tile_pool(name="tpool", bufs=1) as tpool,
            tc.tile_pool(name="cpool", bufs=4) as cpool,
        ):
            nlanes = len(LANES)
            widths = [E * n for _, n in LANES]

            # Persistent per-lane prefix stores: Q(s), s = 2..23 at slot s-2.
            q_re = [qpool.tile([P, 22 * w], F32, tag=f"q_re{li}",
                               name=f"q_re{li}")
                    for li, w in enumerate(widths)]
            q_im = [qpool.tile([P, 22 * w], F32, tag=f"q_im{li}",
                               name=f"q_im{li}")
                    for li, w in enumerate(widths)]

            def qsl(li, s):
                w = widths[li]
                o = (s - 2) * w
                return q_re[li][:, o:o + w], q_im[li][:, o:o + w]

            def load_u(k):
                out = []
                for li, w in enumerate(widths):
                    ur = upool.tile([P, w], F32, tag=f"u_re{li}",
                                    name=f"u_re{li}_{k}")
                    ui = upool.tile([P, w], F32, tag=f"u_im{li}",
                                    name=f"u_im{li}_{k}")
                    us = upool.tile([P, w], F32, tag=f"u_sm{li}",
                                    name=f"u_sm{li}_{k}")
                    nc.sync.dma_start(out=ur[:],
                                      in_=ure_d[k][:, offs[li]:offs[li] + w])
                    nc.sync.dma_start(out=ui[:],
                                      in_=uim_d[k][:, offs[li]:offs[li] + w])
                    nc.sync.dma_start(out=us[:],
                                      in_=usm_d[k][:, offs[li]:offs[li] + w])
                    out.append((ur[:], ui[:], us[:]))
                return out

            def palloc():
                return [(ppool.tile([P, w], F32, tag=f"p_re{li}",
                                    name=f"p_re{li}")[:],
                         ppool.tile([P, w], F32, tag=f"p_im{li}",
                                    name=f"p_im{li}")[:])
                        for li, w in enumerate(widths)]

            def store_p(k, pts):
                for li, w in enumerate(widths):
                    nc.sync.dma_start(out=pre_d[k][:, offs[li]:offs[li] + w],
                                      in_=pts[li][0])
                    nc.sync.dma_start(out=pim_d[k][:, offs[li]:offs[li] + w],
                                      in_=pts[li][1])

            def csum_tile(li, tag):
                w = widths[li]
                return cpool.tile([P, w], F32, tag=f"{tag}{li}",
                                  name=f"{tag}{li}")[:]

            def mm(li, dst, a, b, csum=None):
                name, n = LANES[li]
                _cmatmul(nc, engs[name], tpool, li, n,
                         dst[0], dst[1], a[0], a[1], b[0], b[1],
                         asum=(a[2] if len(a) > 2 else None),
                         bsum=(b[2] if len(b) > 2 else None),
                         csum=csum)

            # ---- prefix pass: Q(k+1) = Q(k) @ V(k) ----
            # cur carries (re, im, sum); Q slices store only re/im.
            cur = None
            for k in range(L):
                u = load_u(k)
                if k == 0:
                    cur = u
                    continue
                if k <= L - 2:
                    nxt = []
                    for li in range(nlanes):
                        qs = csum_tile(li, "qs")
                        dre, dim = qsl(li, k + 1)
                        mm(li, (dre, dim), cur[li], u[li], csum=qs)
                        w = widths[li]
                        nc.sync.dma_start(
                            out=qsm_d[k - 1][:, offs[li]:offs[li] + w],
                            in_=qs)
                        nxt.append((dre, dim, qs))
                    cur = nxt
                else:
                    pts = palloc()                      # Q(24) = P(0)
                    for li in range(nlanes):
                        mm(li, pts[li], cur[li], u[li])
                    store_p(0, pts)

            # ---- suffix pass: S(k) = V(k) @ S(k+1); P(k) = S(k) @ Q(k) ----
            s_cur = None
            for k in range(L - 1, -1, -1):
                u = load_u(k)
                def qtrip(li, s):
                    w = widths[li]
                    bq = cpool.tile([P, w], F32, tag=f"bq{li}",
                                    name=f"bq{li}_{s}")
                    nc.sync.dma_start(
                        out=bq[:],
                        in_=qsm_d[s - 2][:, offs[li]:offs[li] + w])
                    return qsl(li, s) + (bq[:],)

                if k == L - 1:
                    s_cur = u                           # S(23) = V23 (+sum)
                    pts = palloc()
                    for li in range(nlanes):
                        mm(li, pts[li], s_cur[li], qtrip(li, k))
                    store_p(k, pts)
                elif k >= 1:
                    s_new = []
                    for li, w in enumerate(widths):
                        sr = spool.tile([P, w], F32, tag=f"s_re{li}",
                                        name=f"s_re{li}_{k}")
                        si = spool.tile([P, w], F32, tag=f"s_im{li}",
                                        name=f"s_im{li}_{k}")
                        ss = csum_tile(li, "ss")
                        mm(li, (sr[:], si[:]), u[li], s_cur[li], csum=ss)
                        s_new.append((sr[:], si[:], ss))
                    s_cur = s_new
                    if k >= 2:
                        pts = palloc()
                        for li in range(nlanes):
                            mm(li, pts[li], s_cur[li], qtrip(li, k))
                        store_p(k, pts)
                else:
                    # k == 0: P(1) = S(1) @ Q(1), Q(1) = V0
                    pts = palloc()
                    for li in range(nlanes):
                        mm(li, pts[li], s_cur[li], u[li])
                    store_p(1, pts)

    nc.compile()
    _prog_cache["nc"] = nc
    return nc


def _lane_cols():
    cols = []
    lo = 0
    for _, n in LANES:
        cols.append((lo, n))
        lo += n
    return cols


def _canonicalize(U_re, U_im):
    """Full inputs -> per-core input maps (core c: mu = c//2, half = c%2)."""
    cols = _lane_cols()
    in_maps = []
    for c in range(8):
        mu, h = c // 2, c % 2
        m = {}
        for name, U in (("u_re", U_re), ("u_im", U_im)):
            canon = np.moveaxis(U[mu], mu, 0).reshape(L, L**3, E)
            shard = canon[:, h * HALF:(h + 1) * HALF, :]        # [L, 6912, 9]
            sp = shard.reshape(L, P, F, E)
            blocks = [
                np.ascontiguousarray(
                    sp[:, :, lo:lo + n, :].transpose(0, 1, 3, 2)
                ).reshape(L, P, E * n)
                for lo, n in cols
            ]
            m[name] = np.concatenate(blocks, axis=2)            # [L, P, FB]
        m["u_sum"] = m["u_re"] + m["u_im"]
        in_maps.append(m)
    return in_maps


def _assemble(results):
    cols = _lane_cols()
    out = np.empty((2, NMU, L, L, L, L, 3, 3), dtype=np.float32)
    for mu in range(4):
        for ri, name in ((0, "p_re"), (1, "p_im")):
            halves = []
            for h in (0, 1):
                flat = results[2 * mu + h][name]                # [L, P, FB]
                sp = np.empty((L, P, F, E), dtype=np.float32)
                off = 0
                for lo, n in cols:
                    blk = flat[:, :, off:off + E * n].reshape(L, P, E, n)
                    sp[:, :, lo:lo + n, :] = blk.transpose(0, 1, 3, 2)
                    off += E * n
                halves.append(sp.reshape(L, HALF, E))
            canon = np.concatenate(halves, axis=1)              # [24, 13824, 9]
            rest = [d for d in range(4) if d != mu]
            shape = (L,) + tuple(L for _ in rest) + (3, 3)
            arr = canon.reshape(shape)
            out[ri, mu] = np.moveaxis(arr, 0, mu)
    return out


def kernel(U_re, U_im):
    U_re = np.asarray(U_re, dtype=np.float32)
    U_im = np.asarray(U_im, dtype=np.float32)
    nc = build_program()
    in_maps = _canonicalize(U_re, U_im)
    res = run_bass_kernel_spmd(nc, in_maps, core_ids=list(range(8)))
    return _assemble(res.results)


# revision 20
# speedup vs baseline: 1.0036x; 1.0036x over previous
"""Polyakov-loop generator kernel for Trainium2 (8 NeuronCores, SPMD).

Problem: U (complex 3x3 link field) on a [4, 24,24,24,24] lattice.
For each direction mu, every site x: P(x) = prod_{k=0..23} U_mu(x + k*mu_hat)
(periodic).  Output = stack([Re, Im]) of shape [2, 4, 24,24,24,24, 3, 3].

Strategy:
  - core c in 0..7 handles direction mu = c // 2, half h = c % 2 of the
    13824-site batch (the three non-mu lattice axes).
  - host canonicalizes U[mu] per core to [24 slices, 128 partitions, 486]
    (planar: 9 entries x 54 site-columns, entry-major), plus a precomputed
    U_re+U_im plane, so the SPMD program is direction-agnostic.
  - on-chip: all 24 cyclic products per line via prefix/suffix products:
        Q(s) = V0..V(s-1),  S(s) = Vs..V23,  P(s) = S(s) @ Q(s)
    68 batched complex-3x3 matmul steps instead of the naive 23*24.
  - each step is a Gauss-3M complex matmul (3 real bulk products, the 3
    contraction terms fused per instruction via broadcast access patterns),
    with (re+im) operand sums carried along the chains so most steps skip
    the 3M pre-additions.  All elementwise work runs on the Vector engine
    (concurrent GPSIMD measurably poisons DVE throughput via SBUF fabric
    contention).
"""

import sys

sys.path.insert(0, "/opt/trn_rl_repo")

import numpy as np

import concourse.bacc as bacc
import concourse.mybir as mybir
from concourse.tile import TileContext
from concourse.bass_utils import run_bass_kernel_spmd

F32 = mybir.dt.float32
L = 24          # lattice extent (product length)
NMU = 4
P = 128         # SBUF partitions
F = 54          # site-columns per partition (6912 = 128 * 54)
E = 9           # 3x3 entries
FB = F * E      # 486 free elems per slice
HALF = P * F    # 6912 sites per core
MULT = mybir.AluOpType.mult
ADD = mybir.AluOpType.add
SUB = mybir.AluOpType.subtract

# (engine_name, n_site_columns) per independent chain lane.
LANES = (("vector", 54),)

_prog_cache = {}


def _cmatmul(nc, eng, tpool, lane_id, n, cre, cim, are, aim, bre, bim,
             asum=None, bsum=None, csum=None):
    """C = A @ B (complex 3x3, batched) on packed [P, 9n] planar APs.

    Gauss 3M: M1 = Ar@Br, M2 = Ai@Bi, M3 = (Ar+Ai)@(Br+Bi);
    Cre = M1 - M2, Cim = M3 - M1 - M2.
    asum/bsum: optional precomputed (re+im) operand planes.
    csum: optional output AP to receive Cre+Cim (for chain carrying).
    """
    tt = eng.tensor_tensor
    jn = 27 * n
    w = E * n

    def av(x):  # A as [p, ij(9), k-bcast(3), s(n)]
        return (x.rearrange("p (ij s) -> p ij s", ij=9, s=n)
                .unsqueeze(2).broadcast_to([P, 9, 3, n]))

    def bv(x):  # B as [p, i-bcast(3), j(3), ks(3n)]
        return (x.rearrange("p (j ks) -> p j ks", j=3, ks=3 * n)
                .unsqueeze(1).broadcast_to([P, 3, 3, 3 * n]))

    def tv(t):  # product out view [p, ij(9), k(3), s(n)] (contiguous)
        return t.rearrange("p (ij k s) -> p ij k s", ij=9, k=3, s=n)

    def tj(t, j):  # j-th term [p, i(3), k(3), s(n)]
        return t.rearrange("p (i j ks) -> p i j ks",
                           i=3, j=3, ks=3 * n)[:, :, j, :]

    tab = tpool.tile([P, 2 * jn], F32, tag=f"tab{lane_id}",
                     name=f"tab{lane_id}")
    ta, tb = tab[:, :jn], tab[:, jn:]
    tc = tpool.tile([P, jn], F32, tag=f"tc{lane_id}", name=f"tc{lane_id}")
    m12 = tpool.tile([P, 2 * w], F32, tag=f"m12{lane_id}",
                     name=f"m12{lane_id}")
    m1, m2 = m12[:, :w], m12[:, w:]

    def tabj(j):  # j-th terms of both ta and tb: [p, ci(6), ks(3n)]
        return tab[:].rearrange("p (ci j ks) -> p ci j ks",
                                ci=6, j=3, ks=3 * n)[:, :, j, :]

    m12v = m12[:]

    if asum is None:
        at = tpool.tile([P, w], F32, tag=f"as{lane_id}", name=f"as{lane_id}")
        tt(out=at[:], in0=are, in1=aim, op=ADD)
        asum = at[:]
    if bsum is None:
        bt = tpool.tile([P, w], F32, tag=f"bs{lane_id}", name=f"bs{lane_id}")
        tt(out=bt[:], in0=bre, in1=bim, op=ADD)
        bsum = bt[:]

    tt(out=tv(ta), in0=av(are), in1=bv(bre), op=MULT)
    tt(out=tv(tb), in0=av(aim), in1=bv(bim), op=MULT)
    tt(out=tv(tc[:]), in0=av(asum), in1=bv(bsum), op=MULT)
    tt(out=m12v, in0=tabj(0), in1=tabj(1), op=ADD)      # [m1; m2] fold
    tt(out=m12v, in0=m12v, in1=tabj(2), op=ADD)
    tt(out=cim, in0=tj(tc[:], 0), in1=tj(tc[:], 1), op=ADD)
    tt(out=cim, in0=cim, in1=tj(tc[:], 2), op=ADD)
    tt(out=cre, in0=m1, in1=m2, op=SUB)
    tt(out=cim, in0=cim, in1=m1, op=SUB)
    tt(out=cim, in0=cim, in1=m2, op=SUB)
    if csum is not None:
        tt(out=csum, in0=cre, in1=cim, op=ADD)


def build_program():
    if "nc" in _prog_cache:
        return _prog_cache["nc"]

    nc = bacc.Bacc("TRN2", target_bir_lowering=False, debug=False, num_devices=8)
    ure_d = nc.declare_dram_parameter("u_re", [L, P, FB], F32, isOutput=False)
    uim_d = nc.declare_dram_parameter("u_im", [L, P, FB], F32, isOutput=False)
    usm_d = nc.declare_dram_parameter("u_sum", [L, P, FB], F32, isOutput=False)
    pre_d = nc.declare_dram_parameter("p_re", [L, P, FB], F32, isOutput=True)
    pim_d = nc.declare_dram_parameter("p_im", [L, P, FB], F32, isOutput=True)
    qsm_d = nc.dram_tensor("qsum_scratch", [L - 2, P, FB], F32)

    offs = []
    o = 0
    for _, n in LANES:
        offs.append(o)
        o += E * n
    assert o == FB

    with TileContext(nc) as tc:
        engs = {"vector": nc.vector, "gpsimd": nc.gpsimd}
        with (
            tc.tile_pool(name="qpool", bufs=1) as qpool,
            tc.tile_pool(name="upool", bufs=6) as upool,
            tc.tile_pool(name="spool", bufs=3) as spool,
            tc.tile_pool(name="ppool", bufs=6) as ppool,
            tc.tile_pool(name="tpool", bufs=1) as tpool,
            tc.tile_pool(name="cpool", bufs=4) as cpool,
        ):
            nlanes = len(LANES)
            widths = [E * n for _, n in LANES]

            # Persistent per-lane prefix stores: Q(s), s = 2..23 at slot s-2.
            q_re = [qpool.tile([P, 22 * w], F32, tag=f"q_re{li}",
                               name=f"q_re{li}")
                    for li, w in enumerate(widths)]
            q_im = [qpool.tile([P, 22 * w], F32, tag=f"q_im{li}",
                               name=f"q_im{li}")
                    for li, w in enumerate(widths)]

            def qsl(li, s):
                w = widths[li]
                o = (s - 2) * w
                return q_re[li][:, o:o + w], q_im[li][:, o:o + w]

            def load_u(k):
                out = []
                for li, w in enumerate(widths):
                    ur = upool.tile([P, w], F32, tag=f"u_re{li}",
                                    name=f"u_re{li}_{k}")
                    ui = upool.tile([P, w], F32, tag=f"u_im{li}",
                                    name=f"u_im{li}_{k}")
                    us = upool.tile([P, w], F32, tag=f"u_sm{li}",
                                    name=f"u_sm{li}_{k}")
                    nc.sync.dma_start(out=ur[:],
                                      in_=ure_d[k][:, offs[li]:offs[li] + w])
                    nc.sync.dma_start(out=ui[:],
                                      in_=uim_d[k][:, offs[li]:offs[li] + w])
                    nc.sync.dma_start(out=us[:],
                                      in_=usm_d[k][:, offs[li]:offs[li] + w])
                    out.append((ur[:], ui[:], us[:]))
                return out

            def palloc():
                return [(ppool.tile([P, w], F32, tag=f"p_re{li}",
                                    name=f"p_re{li}")[:],
                         ppool.tile([P, w], F32, tag=f"p_im{li}",
                                    name=f"p_im{li}")[:])
                        for li, w in enumerate(widths)]

            def store_p(k, pts):
                for li, w in enumerate(widths):
                    nc.sync.dma_start(out=pre_d[k][:, offs[li]:offs[li] + w],
                                      in_=pts[li][0])
                    nc.sync.dma_start(out=pim_d[k][:, offs[li]:offs[li] + w],
                                      in_=pts[li][1])

            def csum_tile(li, tag):
                w = widths[li]
                return cpool.tile([P, w], F32, tag=f"{tag}{li}",
                                  name=f"{tag}{li}")[:]

            def mm(li, dst, a, b, csum=None):
                name, n = LANES[li]
                _cmatmul(nc, engs[name], tpool, li, n,
                         dst[0], dst[1], a[0], a[1], b[0], b[1],
                         asum=(a[2] if len(a) > 2 else None),
                         bsum=(b[2] if len(b) > 2 else None),
                         csum=csum)

            # ---- prefix pass: Q(k+1) = Q(k) @ V(k) ----
            # cur carries (re, im, sum); Q slices store only re/im.
            cur = None
            for k in range(L):
                u = load_u(k)
                if k == 0:
                    cur = u
                    continue
                if k <= L - 2:
                    nxt = []
                    for li in range(nlanes):
                        qs = csum_tile(li, "qs")
                        dre, dim = qsl(li, k + 1)
                        mm(li, (dre, dim), cur[li], u[li], csum=qs)
                        w = widths[li]
                        nc.sync.dma_start(
                            out=qsm_d[k - 1][:, offs[li]:offs[li] + w],
                            in_=qs)
                        nxt.append((dre, dim, qs))
                    cur = nxt
                else:
                    pts = palloc()                      # Q(24) = P(0)
                    for li in range(nlanes):
                        mm(li, pts[li], cur[li], u[li])
                    store_p(0, pts)

            # ---- suffix pass: S(k) = V(k) @ S(k+1); P(k) = S(k) @ Q(k) ----
            s_cur = None
            for k in range(L - 1, -1, -1):
                u = load_u(k)
                def qtrip(li, s):
                    w = widths[li]
                    bq = cpool.tile([P, w], F32, tag=f"bq{li}",
                                    name=f"bq{li}_{s}")
                    nc.sync.dma_start(
                        out=bq[:],
                        in_=qsm_d[s - 2][:, offs[li]:offs[li] + w])
                    return qsl(li, s) + (bq[:],)

                def emit_pending(pending, b_override=None):
                    s, strip = pending
                    pts = palloc()
                    for li in range(nlanes):
                        b = b_override[li] if b_override else qtrip(li, s)
                        mm(li, pts[li], strip[li], b)
                    store_p(s, pts)

                if k == L - 1:
                    s_cur = u                           # S(23) = V23 (+sum)
                    pending = (k, s_cur)
                elif k >= 1:
                    s_new = []
                    for li, w in enumerate(widths):
                        sr = spool.tile([P, w], F32, tag=f"s_re{li}",
                                        name=f"s_re{li}_{k}")
                        si = spool.tile([P, w], F32, tag=f"s_im{li}",
                                        name=f"s_im{li}_{k}")
                        ss = csum_tile(li, "ss")
                        mm(li, (sr[:], si[:]), u[li], s_cur[li], csum=ss)
                        s_new.append((sr[:], si[:], ss))
                    # combine for the PREVIOUS S (independent of the update
                    # just emitted) — gives the scheduler adjacent
                    # independent work.
                    emit_pending(pending)
                    s_cur = s_new
                    pending = (k, s_cur)
                else:
                    # k == 0: P(1) = S(1) @ Q(1), Q(1) = V0
                    emit_pending(pending, b_override=u)

    nc.compile()
    _prog_cache["nc"] = nc
    return nc


def _lane_cols():
    cols = []
    lo = 0
    for _, n in LANES:
        cols.append((lo, n))
        lo += n
    return cols


def _canonicalize(U_re, U_im):
    """Full inputs -> per-core input maps (core c: mu = c//2, half = c%2)."""
    cols = _lane_cols()
    in_maps = []
    for c in range(8):
        mu, h = c // 2, c % 2
        m = {}
        for name, U in (("u_re", U_re), ("u_im", U_im)):
            canon = np.moveaxis(U[mu], mu, 0).reshape(L, L**3, E)
            shard = canon[:, h * HALF:(h + 1) * HALF, :]        # [L, 6912, 9]
            sp = shard.reshape(L, P, F, E)
            blocks = [
                np.ascontiguousarray(
                    sp[:, :, lo:lo + n, :].transpose(0, 1, 3, 2)
                ).reshape(L, P, E * n)
                for lo, n in cols
            ]
            m[name] = np.concatenate(blocks, axis=2)            # [L, P, FB]
        m["u_sum"] = m["u_re"] + m["u_im"]
        in_maps.append(m)
    return in_maps


def _assemble(results):
    cols = _lane_cols()
    out = np.empty((2, NMU, L, L, L, L, 3, 3), dtype=np.float32)
    for mu in range(4):
        for ri, name in ((0, "p_re"), (1, "p_im")):
            halves = []
            for h in (0, 1):
                flat = results[2 * mu + h][name]                # [L, P, FB]
                sp = np.empty((L, P, F, E), dtype=np.float32)
                off = 0
                for lo, n in cols:
                    blk = flat[:, :, off:off + E * n].reshape(L, P, E, n)
                    sp[:, :, lo:lo + n, :] = blk.transpose(0, 1, 3, 2)
                    off += E * n
                halves.append(sp.reshape(L, HALF, E))
            canon = np.concatenate(halves, axis=1)              # [24, 13824, 9]
            rest = [d for d in range(4) if d != mu]
            shape = (L,) + tuple(L for _ in rest) + (3, 3)
            arr = canon.reshape(shape)
            out[ri, mu] = np.moveaxis(arr, 0, mu)
    return out


def kernel(U_re, U_im):
    U_re = np.asarray(U_re, dtype=np.float32)
    U_im = np.asarray(U_im, dtype=np.float32)
    nc = build_program()
    in_maps = _canonicalize(U_re, U_im)
    res = run_bass_kernel_spmd(nc, in_maps, core_ids=list(range(8)))
    return _assemble(res.results)


# revision 21
# speedup vs baseline: 1.0063x; 1.0027x over previous
"""Polyakov-loop generator kernel for Trainium2 (8 NeuronCores, SPMD).

Problem: U (complex 3x3 link field) on a [4, 24,24,24,24] lattice.
For each direction mu, every site x: P(x) = prod_{k=0..23} U_mu(x + k*mu_hat)
(periodic).  Output = stack([Re, Im]) of shape [2, 4, 24,24,24,24, 3, 3].

Strategy:
  - core c in 0..7 handles direction mu = c // 2, half h = c % 2 of the
    13824-site batch (the three non-mu lattice axes).
  - host canonicalizes U[mu] per core to [24 slices, 128 partitions, 486]
    (planar: 9 entries x 54 site-columns, entry-major), plus a precomputed
    U_re+U_im plane, so the SPMD program is direction-agnostic.
  - on-chip: all 24 cyclic products per line via prefix/suffix products:
        Q(s) = V0..V(s-1),  S(s) = Vs..V23,  P(s) = S(s) @ Q(s)
    68 batched complex-3x3 matmul steps instead of the naive 23*24.
  - each step is a Gauss-3M complex matmul (3 real bulk products, the 3
    contraction terms fused per instruction via broadcast access patterns),
    with (re+im) operand sums carried along the chains so most steps skip
    the 3M pre-additions.  All elementwise work runs on the Vector engine
    (concurrent GPSIMD measurably poisons DVE throughput via SBUF fabric
    contention).
"""

import sys

sys.path.insert(0, "/opt/trn_rl_repo")

import numpy as np

import concourse.bacc as bacc
import concourse.mybir as mybir
from concourse.tile import TileContext
from concourse.bass_utils import run_bass_kernel_spmd

F32 = mybir.dt.float32
L = 24          # lattice extent (product length)
NMU = 4
P = 128         # SBUF partitions
F = 54          # site-columns per partition (6912 = 128 * 54)
E = 9           # 3x3 entries
FB = F * E      # 486 free elems per slice
HALF = P * F    # 6912 sites per core
MULT = mybir.AluOpType.mult
ADD = mybir.AluOpType.add
SUB = mybir.AluOpType.subtract

# (engine_name, n_site_columns) per independent chain lane.
LANES = (("vector", 54),)

_prog_cache = {}


def _cmatmul(nc, eng, tpool, lane_id, n, cre, cim, are, aim, bre, bim,
             asum=None, bsum=None, csum=None):
    """C = A @ B (complex 3x3, batched) on packed [P, 9n] planar APs.

    Gauss 3M: M1 = Ar@Br, M2 = Ai@Bi, M3 = (Ar+Ai)@(Br+Bi);
    Cre = M1 - M2, Cim = M3 - M1 - M2.
    asum/bsum: optional precomputed (re+im) operand planes.
    csum: optional output AP to receive Cre+Cim (for chain carrying).
    """
    tt = eng.tensor_tensor
    jn = 27 * n
    w = E * n

    def av(x):  # A as [p, ij(9), k-bcast(3), s(n)]
        return (x.rearrange("p (ij s) -> p ij s", ij=9, s=n)
                .unsqueeze(2).broadcast_to([P, 9, 3, n]))

    def bv(x):  # B as [p, i-bcast(3), j(3), ks(3n)]
        return (x.rearrange("p (j ks) -> p j ks", j=3, ks=3 * n)
                .unsqueeze(1).broadcast_to([P, 3, 3, 3 * n]))

    def tv(t):  # product out view [p, ij(9), k(3), s(n)] (contiguous)
        return t.rearrange("p (ij k s) -> p ij k s", ij=9, k=3, s=n)

    def tj(t, j):  # j-th term [p, i(3), k(3), s(n)]
        return t.rearrange("p (i j ks) -> p i j ks",
                           i=3, j=3, ks=3 * n)[:, :, j, :]

    tab = tpool.tile([P, 2 * jn], F32, tag=f"tab{lane_id}",
                     name=f"tab{lane_id}")
    ta, tb = tab[:, :jn], tab[:, jn:]
    tc = tpool.tile([P, jn], F32, tag=f"tc{lane_id}", name=f"tc{lane_id}")
    m12 = tpool.tile([P, 2 * w], F32, tag=f"m12{lane_id}",
                     name=f"m12{lane_id}")
    m1, m2 = m12[:, :w], m12[:, w:]

    def tabj(j):  # j-th terms of both ta and tb: [p, ci(6), ks(3n)]
        return tab[:].rearrange("p (ci j ks) -> p ci j ks",
                                ci=6, j=3, ks=3 * n)[:, :, j, :]

    m12v = m12[:]

    if asum is None:
        at = tpool.tile([P, w], F32, tag=f"as{lane_id}", name=f"as{lane_id}")
        tt(out=at[:], in0=are, in1=aim, op=ADD)
        asum = at[:]
    if bsum is None:
        bt = tpool.tile([P, w], F32, tag=f"bs{lane_id}", name=f"bs{lane_id}")
        tt(out=bt[:], in0=bre, in1=bim, op=ADD)
        bsum = bt[:]

    tt(out=tv(ta), in0=av(are), in1=bv(bre), op=MULT)
    tt(out=tv(tb), in0=av(aim), in1=bv(bim), op=MULT)
    tt(out=tv(tc[:]), in0=av(asum), in1=bv(bsum), op=MULT)
    tt(out=m12v, in0=tabj(0), in1=tabj(1), op=ADD)      # [m1; m2] fold
    tt(out=m12v, in0=m12v, in1=tabj(2), op=ADD)
    tt(out=cim, in0=tj(tc[:], 0), in1=tj(tc[:], 1), op=ADD)
    tt(out=cim, in0=cim, in1=tj(tc[:], 2), op=ADD)
    tt(out=cre, in0=m1, in1=m2, op=SUB)
    tt(out=cim, in0=cim, in1=m1, op=SUB)
    tt(out=cim, in0=cim, in1=m2, op=SUB)
    if csum is not None:
        tt(out=csum, in0=cre, in1=cim, op=ADD)


def build_program():
    if "nc" in _prog_cache:
        return _prog_cache["nc"]

    nc = bacc.Bacc("TRN2", target_bir_lowering=False, debug=False, num_devices=8)
    ure_d = nc.declare_dram_parameter("u_re", [L, P, FB], F32, isOutput=False)
    uim_d = nc.declare_dram_parameter("u_im", [L, P, FB], F32, isOutput=False)
    usm_d = nc.declare_dram_parameter("u_sum", [L, P, FB], F32, isOutput=False)
    pre_d = nc.declare_dram_parameter("p_re", [L, P, FB], F32, isOutput=True)
    pim_d = nc.declare_dram_parameter("p_im", [L, P, FB], F32, isOutput=True)
    qsm_d = nc.dram_tensor("qsum_scratch", [L - 2, P, FB], F32)

    offs = []
    o = 0
    for _, n in LANES:
        offs.append(o)
        o += E * n
    assert o == FB

    with TileContext(nc) as tc:
        engs = {"vector": nc.vector, "gpsimd": nc.gpsimd}
        with (
            tc.tile_pool(name="qpool", bufs=1) as qpool,
            tc.tile_pool(name="upool", bufs=6) as upool,
            tc.tile_pool(name="spool", bufs=3) as spool,
            tc.tile_pool(name="ppool", bufs=6) as ppool,
            tc.tile_pool(name="tpool", bufs=1) as tpool,
            tc.tile_pool(name="cpool", bufs=4) as cpool,
        ):
            nlanes = len(LANES)
            widths = [E * n for _, n in LANES]

            # Persistent per-lane prefix stores: Q(s), s = 2..23 at slot s-2.
            q_re = [qpool.tile([P, 22 * w], F32, tag=f"q_re{li}",
                               name=f"q_re{li}")
                    for li, w in enumerate(widths)]
            q_im = [qpool.tile([P, 22 * w], F32, tag=f"q_im{li}",
                               name=f"q_im{li}")
                    for li, w in enumerate(widths)]

            def qsl(li, s):
                w = widths[li]
                o = (s - 2) * w
                return q_re[li][:, o:o + w], q_im[li][:, o:o + w]

            def load_u(k):
                out = []
                for li, w in enumerate(widths):
                    ur = upool.tile([P, w], F32, tag=f"u_re{li}",
                                    name=f"u_re{li}_{k}")
                    ui = upool.tile([P, w], F32, tag=f"u_im{li}",
                                    name=f"u_im{li}_{k}")
                    us = upool.tile([P, w], F32, tag=f"u_sm{li}",
                                    name=f"u_sm{li}_{k}")
                    nc.sync.dma_start(out=ur[:],
                                      in_=ure_d[k][:, offs[li]:offs[li] + w])
                    nc.sync.dma_start(out=ui[:],
                                      in_=uim_d[k][:, offs[li]:offs[li] + w])
                    nc.sync.dma_start(out=us[:],
                                      in_=usm_d[k][:, offs[li]:offs[li] + w])
                    out.append((ur[:], ui[:], us[:]))
                return out

            def palloc():
                return [(ppool.tile([P, w], F32, tag=f"p_re{li}",
                                    name=f"p_re{li}")[:],
                         ppool.tile([P, w], F32, tag=f"p_im{li}",
                                    name=f"p_im{li}")[:])
                        for li, w in enumerate(widths)]

            def store_p(k, pts):
                for li, w in enumerate(widths):
                    nc.sync.dma_start(out=pre_d[k][:, offs[li]:offs[li] + w],
                                      in_=pts[li][0])
                    nc.sync.dma_start(out=pim_d[k][:, offs[li]:offs[li] + w],
                                      in_=pts[li][1])

            def csum_tile(li, tag):
                w = widths[li]
                return cpool.tile([P, w], F32, tag=f"{tag}{li}",
                                  name=f"{tag}{li}")[:]

            def mm(li, dst, a, b, csum=None):
                name, n = LANES[li]
                _cmatmul(nc, engs[name], tpool, li, n,
                         dst[0], dst[1], a[0], a[1], b[0], b[1],
                         asum=(a[2] if len(a) > 2 else None),
                         bsum=(b[2] if len(b) > 2 else None),
                         csum=csum)

            # ---- prefix pass: Q(k+1) = Q(k) @ V(k) ----
            # cur carries (re, im, sum); Q slices store only re/im.
            cur = None
            for k in range(L):
                u = load_u(k)
                if k == 0:
                    cur = u
                    continue
                if k <= L - 2:
                    nxt = []
                    for li in range(nlanes):
                        qs = csum_tile(li, "qs")
                        dre, dim = qsl(li, k + 1)
                        mm(li, (dre, dim), cur[li], u[li], csum=qs)
                        w = widths[li]
                        nc.sync.dma_start(
                            out=qsm_d[k - 1][:, offs[li]:offs[li] + w],
                            in_=qs)
                        nxt.append((dre, dim, qs))
                    cur = nxt
                else:
                    pts = palloc()                      # Q(24) = P(0)
                    for li in range(nlanes):
                        mm(li, pts[li], cur[li], u[li])
                    store_p(0, pts)

            # ---- suffix pass: S(k) = V(k) @ S(k+1); P(k) = S(k) @ Q(k) ----
            s_cur = None
            for k in range(L - 1, -1, -1):
                u = load_u(k)
                def qtrip(li, s):
                    w = widths[li]
                    bq = cpool.tile([P, w], F32, tag=f"bq{li}",
                                    name=f"bq{li}_{s}")
                    nc.sync.dma_start(
                        out=bq[:],
                        in_=qsm_d[s - 2][:, offs[li]:offs[li] + w])
                    return qsl(li, s) + (bq[:],)

                def emit_pending(pending, b_override=None):
                    s, strip = pending
                    pts = palloc()
                    for li in range(nlanes):
                        b = b_override[li] if b_override else qtrip(li, s)
                        mm(li, pts[li], strip[li], b)
                    store_p(s, pts)

                if k == L - 1:
                    s_cur = u                           # S(23) = V23 (+sum)
                    pending = (k, s_cur)
                elif k >= 1:
                    s_new = []
                    for li, w in enumerate(widths):
                        sr = spool.tile([P, w], F32, tag=f"s_re{li}",
                                        name=f"s_re{li}_{k}")
                        si = spool.tile([P, w], F32, tag=f"s_im{li}",
                                        name=f"s_im{li}_{k}")
                        ss = csum_tile(li, "ss")
                        mm(li, (sr[:], si[:]), u[li], s_cur[li], csum=ss)
                        s_new.append((sr[:], si[:], ss))
                    # combine for the PREVIOUS S (independent of the update
                    # just emitted) — gives the scheduler adjacent
                    # independent work.
                    emit_pending(pending)
                    s_cur = s_new
                    pending = (k, s_cur)
                else:
                    # k == 0: P(1) = S(1) @ Q(1), Q(1) = V0
                    emit_pending(pending, b_override=u)

    nc.compile()
    _prog_cache["nc"] = nc
    return nc


def _lane_cols():
    cols = []
    lo = 0
    for _, n in LANES:
        cols.append((lo, n))
        lo += n
    return cols


def _canonicalize(U_re, U_im):
    """Full inputs -> per-core input maps (core c: mu = c//2, half = c%2)."""
    cols = _lane_cols()
    in_maps = []
    for c in range(8):
        mu, h = c // 2, c % 2
        m = {}
        for name, U in (("u_re", U_re), ("u_im", U_im)):
            canon = np.moveaxis(U[mu], mu, 0).reshape(L, L**3, E)
            shard = canon[:, h * HALF:(h + 1) * HALF, :]        # [L, 6912, 9]
            sp = shard.reshape(L, P, F, E)
            blocks = [
                np.ascontiguousarray(
                    sp[:, :, lo:lo + n, :].transpose(0, 1, 3, 2)
                ).reshape(L, P, E * n)
                for lo, n in cols
            ]
            m[name] = np.concatenate(blocks, axis=2)            # [L, P, FB]
        m["u_sum"] = m["u_re"] + m["u_im"]
        in_maps.append(m)
    return in_maps


def _assemble(results):
    cols = _lane_cols()
    out = np.empty((2, NMU, L, L, L, L, 3, 3), dtype=np.float32)
    for mu in range(4):
        for ri, name in ((0, "p_re"), (1, "p_im")):
            halves = []
            for h in (0, 1):
                flat = results[2 * mu + h][name]                # [L, P, FB]
                sp = np.empty((L, P, F, E), dtype=np.float32)
                off = 0
                for lo, n in cols:
                    blk = flat[:, :, off:off + E * n].reshape(L, P, E, n)
                    sp[:, :, lo:lo + n, :] = blk.transpose(0, 1, 3, 2)
                    off += E * n
                halves.append(sp.reshape(L, HALF, E))
            canon = np.concatenate(halves, axis=1)              # [24, 13824, 9]
            rest = [d for d in range(4) if d != mu]
            shape = (L,) + tuple(L for _ in rest) + (3, 3)
            arr = canon.reshape(shape)
            out[ri, mu] = np.moveaxis(arr, 0, mu)
    return out


def kernel(U_re, U_im):
    import time

    U_re = np.asarray(U_re, dtype=np.float32)
    U_im = np.asarray(U_im, dtype=np.float32)
    nc = build_program()
    in_maps = _canonicalize(U_re, U_im)
    last_err = None
    for attempt in range(4):
        if attempt:
            # transient device wedges (NRT_EXEC_UNIT_UNRECOVERABLE) clear
            # after a reinit + short wait
            time.sleep(45 * attempt)
        try:
            res = run_bass_kernel_spmd(nc, in_maps, core_ids=list(range(8)))
            return _assemble(res.results)
        except Exception as e:  # noqa: BLE001
            last_err = e
    raise last_err
